# revision 1
# baseline (speedup 1.0000x reference)
"""2-layer dense GCN on 8 Trainium2 NeuronCores.

Reference computation (all fp32):
    H0 = relu((A_norm @ X) @ W0)
    H1 = relu((A_norm @ H0) @ W1)
A_norm: [16384, 16384], X: [16384, 128], W0/W1: [128, 128].

Sharding: 1D row partition of A_norm (2048 rows/core). Each core holds
A[rows_c].T (host-transposed so the node-contraction dim lands on SBUF
partitions), computes its row block of each layer, and the hidden state
is exchanged between layers with chunked on-device AllGathers.

Device layout is transpose-free:
  - aggregate:  psum[d, i] += X_tile[j, d].T @ A_T_tile[j, i]
                (lhsT = stationary node-major X/H tile, rhs = A^T slice)
  - linear:     psum[i, e]  = M^T_tile[d, i].T @ W[d, e]   (node-major out)
  - relu fused into the PSUM->SBUF eviction on the scalar engine.

The aggregation runs CHUNK-MAJOR (one 512-wide output chunk at a time,
full contraction each): chunk k's hidden tiles finish at ~(k+1)/4 of the
layer, so AllGather k overlaps the remaining chunks' compute — only the
last AllGather is exposed at the layer boundary. The stationary H layout
in SBUF ([128, 512] pieces) is exactly what the chunked AllGathers
produce, so no transposes are needed anywhere.

PRECISION modes:
  - "fp32":   exact fp32 matmuls (4 cyc/row on the PE).
  - "split3": A and X/H split into bf16 hi+lo; aggregate computed as
              Ah@Xh + Al@Xh + Ah@Xl (3 bf16 passes, ~2.5e-6 rel err —
              fp32-class).
  - "bf16":   plain bf16 aggregate (1 cyc/row, half the DMA bytes,
              ~1.1e-3 rel err).
"""

import sys
from contextlib import ExitStack

if "/opt/trn_rl_repo" not in sys.path:
    sys.path.insert(0, "/opt/trn_rl_repo")

import numpy as np

N_NODES = 16384
D = 128
NCORES = 8
ROWS = N_NODES // NCORES  # 2048

PRECISION = "bf16"  # "fp32" | "split3" | "bf16"


def _geom(n_nodes=N_NODES, ncores=NCORES, precision=PRECISION):
    esz = 4 if precision == "fp32" else 2
    nsplit = 2 if precision == "split3" else 1  # hi/lo operand copies
    rows = n_nodes // ncores
    jt = n_nodes // 128          # total j-tiles (contraction tiles)
    jt_per_rank = jt // ncores   # j-tiles covered by one rank's nodes
    ic = min(512, rows)          # i-chunk width (one PSUM bank, fp32 out)
    nch = rows // ic             # i-chunks per core
    # j-tiles per A DMA: ~2 MiB per transfer; deep buffer pool so the
    # A-stream prefetch (~16 MiB) covers the inter-layer AllGather window
    target = 2 * 1024 * 1024
    jg = max(1, target // (128 * rows * esz))
    jg = min(jg, jt)
    while jt % jg:
        jg -= 1
    a_bufs = {"bf16": 8, "fp32": 6, "split3": 3}[precision]
    return dict(
        esz=esz, nsplit=nsplit, rows=rows, jt=jt, jt_per_rank=jt_per_rank,
        ic=ic, nch=nch, jg=jg, ndma=jt // jg, a_bufs=a_bufs,
    )


def build_gcn(n_nodes=N_NODES, d=D, ncores=NCORES, precision=PRECISION):
    """Build the SPMD Bass program (one program, runs on all cores)."""
    import concourse.bass as bass  # noqa: F401
    import concourse.tile as tile
    from concourse import bacc, mybir

    F32 = mybir.dt.float32
    BF16 = mybir.dt.bfloat16
    agg_dt = F32 if precision == "fp32" else BF16

    g_ = _geom(n_nodes, ncores, precision)
    nsplit, rows, jt = g_["nsplit"], g_["rows"], g_["jt"]
    jt_per_rank, ic, nch = g_["jt_per_rank"], g_["ic"], g_["nch"]
    jg, ndma, a_bufs = g_["jg"], g_["ndma"], g_["a_bufs"]
    lt = ic // 128               # linear i-tiles (and h tiles) per chunk

    nc = bacc.Bacc("TRN2", target_bir_lowering=False, num_devices=ncores)

    # A^T shards, host pre-tiled: DMA group g is the contiguous block
    # a_in[g*128 : (g+1)*128, :], covering j-tiles [g*jg, (g+1)*jg) x
    # all output columns, with a_in[g*128+p, t*rows+i] = A^T[(g*jg+t)*128+p, i]
    a_in = [
        nc.dram_tensor(
            f"a{s}", [ndma * 128, jg * rows], agg_dt, kind="ExternalInput"
        )
        for s in range(nsplit)
    ]
    # x_t: X pre-tiled on host into the AllGather layout:
    # x_t[r*128 + p, tl*128 + dd] = X[(r*jt_per_rank + tl)*128 + p, dd]
    x_in = [
        nc.dram_tensor(f"x{s}", [ncores * 128, rows], agg_dt, kind="ExternalInput")
        for s in range(nsplit)
    ]
    w0 = nc.dram_tensor("w0", [d, d], F32, kind="ExternalInput")
    w1 = nc.dram_tensor("w1", [d, d], F32, kind="ExternalInput")
    h_out = nc.dram_tensor("h_out", [rows, d], F32, kind="ExternalOutput")

    relu = mybir.ActivationFunctionType.Relu

    with tile.TileContext(nc) as tc, ExitStack() as ctx:
        sb1 = ctx.enter_context(tc.tile_pool(name="sb1", bufs=1))
        stat_pool = ctx.enter_context(
            tc.tile_pool(name="stat", bufs=ncores * nsplit)
        )
        a_pool = ctx.enter_context(tc.tile_pool(name="a", bufs=a_bufs))
        m_pool = ctx.enter_context(tc.tile_pool(name="m", bufs=2))
        h_pool = ctx.enter_context(tc.tile_pool(name="h", bufs=4))
        split_pool = ctx.enter_context(tc.tile_pool(name="spl", bufs=4))
        agg_pool = ctx.enter_context(tc.tile_pool(name="agg", bufs=4, space="PSUM"))
        lin_pool = ctx.enter_context(tc.tile_pool(name="lin", bufs=2, space="PSUM"))
        dram = ctx.enter_context(tc.tile_pool(name="dram", bufs=1, space="DRAM"))

        w0_sb = sb1.tile([d, d], F32)
        nc.scalar.dma_start(out=w0_sb[:], in_=w0[:])
        w1_sb = sb1.tile([d, d], F32)
        nc.scalar.dma_start(out=w1_sb[:], in_=w1[:])

        def load_stat_chunks(srcs, lname):
            """srcs: per split s: [ncores*128, rows] DRAM view.
            Returns stat[s][r] = [128, rows] SBUF tile."""
            out = []
            for s in range(nsplit):
                chunks = []
                for r in range(ncores):
                    sc = stat_pool.tile(
                        [128, rows], agg_dt, name=f"{lname}{s}_{r}", tag="sc"
                    )
                    nc.gpsimd.dma_start(
                        out=sc[:], in_=srcs[s][r * 128 : (r + 1) * 128, :]
                    )
                    chunks.append(sc)
                out.append(chunks)
            return out

        def layer(stat, w_sb, write_out, layer_done):
            # stat[s][r]: stationary chunks; j-tile j lives in chunk
            # r=j//jt_per_rank at cols (j%jt_per_rank)*128
            passes = [(0, 0)] if nsplit == 1 else [(0, 0), (1, 0), (0, 1)]
            agg = [
                agg_pool.tile([128, ic], F32, name=f"ps{c}", tag="ps")
                for c in range(nch)
            ]
            for g in range(ndma):
                ats = []
                for s in range(nsplit):
                    at = a_pool.tile(
                        [128, jg * rows], agg_dt, name=f"at{s}", tag=f"at{s}"
                    )
                    eng = nc.sync if (g + s) % 2 == 0 else nc.scalar
                    eng.dma_start(
                        out=at[:], in_=a_in[s][g * 128 : (g + 1) * 128, :]
                    )
                    ats.append(at)
                for t in range(jg):
                    j = g * jg + t
                    jr = j % jt_per_rank
                    for pi, (ls, rs) in enumerate(passes):
                        lhs = stat[ls][j // jt_per_rank][
                            :, jr * 128 : (jr + 1) * 128
                        ]
                        for c in range(nch):
                            nc.tensor.matmul(
                                agg[c][:],
                                lhsT=lhs,
                                rhs=ats[rs][
                                    :, t * rows + c * ic : t * rows + (c + 1) * ic
                                ],
                                start=(j == 0 and pi == 0),
                                stop=(j == jt - 1 and pi == len(passes) - 1),
                            )
            # linear + relu, node-major output tiles
            for c in range(nch):
                mt = m_pool.tile([128, ic], F32, name="mt", tag="mt")
                nc.vector.tensor_copy(out=mt[:], in_=agg[c][:])
                for it in range(lt):
                    lp = lin_pool.tile([128, d], F32, name="lp", tag="lp")
                    nc.tensor.matmul(
                        lp[:],
                        lhsT=mt[:, it * 128 : (it + 1) * 128],
                        rhs=w_sb[:],
                        start=True,
                        stop=True,
                    )
                    ht = h_pool.tile([128, d], F32, name="ht", tag="ht")
                    nc.scalar.activation(ht[:], lp[:], relu)
                    write_out(c, it, ht)
            layer_done()

        # ---- layer 0 ----
        stat0 = load_stat_chunks([x[:] for x in x_in], "sx")
        # packed hidden-state bounce ([hi | lo] along free dim when split)
        h_tb = dram.tile([128, nsplit * rows], agg_dt, name="h_tb")
        h_ag = dram.tile(
            [ncores * 128, nsplit * rows], agg_dt, addr_space="Shared", name="h_ag"
        )

        def write_l0(c, it, ht):
            tl = c * lt + it
            if precision == "fp32":
                nc.scalar.dma_start(
                    out=h_tb[:, tl * 128 : (tl + 1) * 128], in_=ht[:]
                )
                return
            hh = split_pool.tile([128, d], BF16, name="hh", tag="hh")
            nc.vector.tensor_copy(out=hh[:], in_=ht[:])
            nc.scalar.dma_start(out=h_tb[:, tl * 128 : (tl + 1) * 128], in_=hh[:])
            if nsplit == 2:
                hh32 = split_pool.tile([128, d], F32, name="hh32", tag="hh32")
                nc.vector.tensor_copy(out=hh32[:], in_=hh[:])
                hl = split_pool.tile([128, d], BF16, name="hl", tag="hl")
                nc.vector.tensor_sub(out=hl[:], in0=ht[:], in1=hh32[:])
                nc.scalar.dma_start(
                    out=h_tb[:, rows + tl * 128 : rows + (tl + 1) * 128], in_=hl[:]
                )

        def ag_l0():
            import concourse.mybir as _mb

            nc.gpsimd.collective_compute(
                "AllGather",
                _mb.AluOpType.bypass,
                replica_groups=[list(range(ncores))],
                ins=[h_tb[:]],
                outs=[h_ag[:]],
            )

        layer(stat0, w0_sb, write_l0, ag_l0)

        # ---- layer 1 ----
        stat1 = load_stat_chunks(
            [h_ag[:, s * rows : (s + 1) * rows] for s in range(nsplit)], "sh"
        )

        def write_l1(c, it, ht):
            nc.scalar.dma_start(
                out=h_out[c * ic + it * 128 : c * ic + (it + 1) * 128, :], in_=ht[:]
            )

        layer(stat1, w1_sb, write_l1, lambda: None)

    nc.finalize()
    return nc


def _tile_stat(X, ncores, jt_per_rank):
    rows = jt_per_rank * 128
    return np.ascontiguousarray(
        X.reshape(ncores, jt_per_rank, 128, D).transpose(0, 2, 1, 3)
        .reshape(ncores * 128, rows)
    )


def shard_inputs(A_norm, X, n_nodes=N_NODES, ncores=NCORES, precision=PRECISION):
    """Host-side shard prep. Returns per-core input maps."""
    import ml_dtypes

    bf16 = ml_dtypes.bfloat16
    g_ = _geom(n_nodes, ncores, precision)
    rows, jt_per_rank = g_["rows"], g_["jt_per_rank"]
    jg, ndma = g_["jg"], g_["ndma"]

    def tile_a(a_tc):
        # [n_nodes, rows] -> [ndma*128, jg*rows] so DMA group g is the
        # contiguous block a_pre[g*128:(g+1)*128, :] with
        # a_pre[g*128+p, t*rows+i] = a_tc[(g*jg+t)*128+p, i]
        return np.ascontiguousarray(
            a_tc.reshape(ndma, jg, 128, rows).swapaxes(1, 2)
            .reshape(ndma * 128, jg * rows)
        )

    x_t = _tile_stat(X, ncores, jt_per_rank)
    if precision == "fp32":
        xs = [x_t]
    else:
        x_hi = x_t.astype(bf16)
        xs = [x_hi]
        if precision == "split3":
            xs.append((x_t - x_hi.astype(np.float32)).astype(bf16))

    in_maps = []
    for c in range(ncores):
        a_tc = np.ascontiguousarray(A_norm[c * rows : (c + 1) * rows, :].T)
        m = {}
        if precision == "fp32":
            m["a0"] = tile_a(a_tc)
        else:
            a_hi = a_tc.astype(bf16)
            m["a0"] = tile_a(a_hi)
            if precision == "split3":
                m["a1"] = tile_a((a_tc - a_hi.astype(np.float32)).astype(bf16))
        for s, x in enumerate(xs):
            m[f"x{s}"] = x
        in_maps.append(m)
    return in_maps


_CACHED = {}


def kernel(A_norm, X, W0, W1):
    A_norm = np.ascontiguousarray(A_norm, dtype=np.float32)
    X = np.ascontiguousarray(X, dtype=np.float32)
    W0 = np.ascontiguousarray(W0, dtype=np.float32)
    W1 = np.ascontiguousarray(W1, dtype=np.float32)

    from concourse.bass_utils import run_bass_kernel_spmd

    if PRECISION not in _CACHED:
        _CACHED[PRECISION] = build_gcn(precision=PRECISION)
    nc = _CACHED[PRECISION]

    in_maps = shard_inputs(A_norm, X, precision=PRECISION)
    for m in in_maps:
        m["w0"] = W0
        m["w1"] = W1

    res = run_bass_kernel_spmd(nc, in_maps, core_ids=list(range(NCORES)))
    return np.concatenate([res.results[c]["h_out"] for c in range(NCORES)], axis=0)



# revision 7
# speedup vs baseline: 1.4902x; 1.4902x over previous
"""2-layer dense GCN on 8 Trainium2 NeuronCores — fp8 A-stream version.

Reference computation (all fp32):
    H0 = relu((A_norm @ X) @ W0)
    H1 = relu((A_norm @ H0) @ W1)
A_norm: [16384, 16384], X: [16384, 128], W0/W1: [128, 128].

Sharding: 1D row partition of A_norm (2048 rows/core). Each core holds
A[rows_c].T host-quantized to fp8-e3m4 (x 2^16 scale, folded back via
W/2^16), streamed over HBM at 1 byte/element. The stationary X/H tiles
stay bf16; the PE runs mixed-dtype matmuls (bf16 lhsT x e3m4 rhs),
which keeps end-to-end error at the bf16 level (~1.3e-3) while halving
the dominant DMA traffic.

Schedule (per layer, per core): CHUNK-MAJOR. The 2048 output rows are
split into 4 chunks of 512 (one PSUM bank each). A^T is host-tiled in
chunk-major column order so each chunk's full 16384-deep contraction
streams 8 MiB of A and finishes at ~(k+1)/4 of the layer. Chunk k's
hidden tile is AllGathered immediately, overlapping the remaining
chunks' compute — only a slice of the last gather is exposed. Layer 1
consumes the gathered H in piece-major j-order (all ranks' piece-0
tiles first), so it only waits on gather chunk p right before its
piece-p matmuls.

Aggregate matmul:  psum[d, i] += H_tile[q, d].T @ A^T[q, i]
Linear matmul:     psum[i, e]  = M[d, i-slice].T @ W[d, e]  (fp32r)
Relu fused into the PSUM eviction on the scalar engine.
"""

import sys
from contextlib import ExitStack

if "/opt/trn_rl_repo" not in sys.path:
    sys.path.insert(0, "/opt/trn_rl_repo")

import numpy as np

N_NODES = 16384
D = 128
NCORES = 8
ROWS = N_NODES // NCORES   # 2048 output rows per core
NCH = 4                    # output column chunks (512 wide, 1 PSUM bank)
NPIECE = 4                 # j-piece blocks per rank (gather granularity)
NT = 4                     # j-tiles per piece
IC = 512                   # chunk width
NG = NCH * NPIECE          # A DMA groups per layer (2 MiB each)
GW = NCORES * NT * IC      # 16384 columns per A group
S_A = float(2 ** 16)       # fp8 pre-scale for A (max |A|*S_A ~ 8.1)

PRECISION = "fp8mix"


def build_gcn():
    """Build the SPMD Bass program (one program, runs on all cores)."""
    import concourse.bass as bass  # noqa: F401
    import concourse.tile as tile
    from concourse import bacc, mybir

    F32 = mybir.dt.float32
    F32R = mybir.dt.float32r
    BF16 = mybir.dt.bfloat16
    E3 = mybir.dt.float8e3
    relu = mybir.ActivationFunctionType.Relu

    nc = bacc.Bacc("TRN2", target_bir_lowering=False, num_devices=NCORES)

    # A^T shard, host pre-tiled chunk-major (see shard_inputs):
    # group g=(ch*NPIECE+p) is rows [g*128, (g+1)*128) with
    # a_in[g*128+q, (r*NT+t)*IC + i] = A^T[(r*16+p*NT+t)*128+q, ch*IC+i]
    a_in = nc.dram_tensor("a0", [NG * 128, GW], E3, kind="ExternalInput")
    # x_in[r*128+p, tl*128+dd] = X[(r*16+tl)*128+p, dd]  (bf16)
    x_in = nc.dram_tensor("x0", [NCORES * 128, ROWS], BF16, kind="ExternalInput")
    w0 = nc.dram_tensor("w0", [D, D], F32R, kind="ExternalInput")
    w1 = nc.dram_tensor("w1", [D, D], F32R, kind="ExternalInput")
    h_out = nc.dram_tensor("h_out", [ROWS, D], F32, kind="ExternalOutput")

    with tile.TileContext(nc) as tc, ExitStack() as ctx:
        sb1 = ctx.enter_context(tc.tile_pool(name="sb1", bufs=1))
        statx_pool = ctx.enter_context(tc.tile_pool(name="sx", bufs=NCORES))
        stath_pool = ctx.enter_context(
            tc.tile_pool(name="sh", bufs=NCORES * NPIECE)
        )
        a_pool = ctx.enter_context(tc.tile_pool(name="a", bufs=5))
        m_pool = ctx.enter_context(tc.tile_pool(name="m", bufs=2))
        h_pool = ctx.enter_context(tc.tile_pool(name="h", bufs=4))
        agg_pool = ctx.enter_context(tc.tile_pool(name="agg", bufs=4, space="PSUM"))
        lin_pool = ctx.enter_context(tc.tile_pool(name="lin", bufs=2, space="PSUM"))
        dram = ctx.enter_context(tc.tile_pool(name="dram", bufs=1, space="DRAM"))

        w0_sb = sb1.tile([D, D], F32R)
        nc.scalar.dma_start(out=w0_sb[:], in_=w0[:])
        w1_sb = sb1.tile([D, D], F32R)
        nc.scalar.dma_start(out=w1_sb[:], in_=w1[:])

        statx = []
        for r in range(NCORES):
            t_ = statx_pool.tile([128, ROWS], BF16, name=f"sx{r}", tag="sx")
            nc.gpsimd.dma_start(out=t_[:], in_=x_in[r * 128 : (r + 1) * 128, :])
            statx.append(t_)

        # hidden-state bounce + gather buffers, one per chunk
        h_tb = [dram.tile([128, IC], BF16, name=f"htb{c}") for c in range(NCH)]
        h_ag = [
            dram.tile([NCORES * 128, IC], BF16, addr_space="Shared", name=f"hag{c}")
            for c in range(NCH)
        ]
        # gathered H as stationary tiles, one per (rank, piece)
        stath = [
            [
                stath_pool.tile([128, IC], BF16, name=f"sh{r}_{p}", tag="sh")
                for p in range(NPIECE)
            ]
            for r in range(NCORES)
        ]

        def lhs_l0(r, p, t):
            jr = p * NT + t
            return statx[r][:, jr * 128 : (jr + 1) * 128]

        def lhs_l1(r, p, t):
            return stath[r][p][:, t * 128 : (t + 1) * 128]

        def layer(lhs_slice, w_sb, write_out, chunk_done, piece_outer):
            # schedule: L0 chunk-outer (finish chunks early -> early gathers);
            # L1 piece-outer (piece-p rounds start ~1/4-layer apart, so each
            # gather chunk has a late deadline and never stalls the PE)
            if piece_outer:
                sched = [(ch, p) for p in range(NPIECE) for ch in range(NCH)]
            else:
                sched = [(ch, p) for ch in range(NCH) for p in range(NPIECE)]
            aggs = {}
            neng = 0
            for ch, p in sched:
                if p == 0:
                    aggs[ch] = agg_pool.tile([128, IC], F32, name=f"agg{ch}", tag="agg")
                agg = aggs[ch]
                g = ch * NPIECE + p
                at = a_pool.tile([128, GW], E3, name="at", tag="at")
                eng = nc.sync if neng % 2 == 0 else nc.scalar
                neng += 1
                eng.dma_start(out=at[:], in_=a_in[g * 128 : (g + 1) * 128, :])
                for r in range(NCORES):
                    for t in range(NT):
                        nc.tensor.matmul(
                            agg[:],
                            lhsT=lhs_slice(r, p, t),
                            rhs=at[:, (r * NT + t) * IC : (r * NT + t + 1) * IC],
                            start=(p == 0 and r == 0 and t == 0),
                            stop=(p == NPIECE - 1 and r == NCORES - 1 and t == NT - 1),
                        )
                if p == NPIECE - 1:
                    mt = m_pool.tile([128, IC], F32R, name="mt", tag="mt")
                    nc.vector.tensor_copy(out=mt[:], in_=agg[:])
                    for it in range(IC // 128):
                        lp = lin_pool.tile([128, D], F32, name="lp", tag="lp")
                        nc.tensor.matmul(
                            lp[:],
                            lhsT=mt[:, it * 128 : (it + 1) * 128],
                            rhs=w_sb[:],
                            start=True,
                            stop=True,
                        )
                        write_out(ch, it, lp)
                    chunk_done(ch)

        # ---- layer 0 ----
        def write_l0(ch, it, lp):
            ht = h_pool.tile([128, D], BF16, name="ht0", tag="ht0")
            nc.scalar.activation(ht[:], lp[:], relu)
            # gpsimd queue: keeps the latency-sensitive h write off the
            # A-stream queues so the gather triggers early
            nc.gpsimd.dma_start(out=h_tb[ch][:, it * 128 : (it + 1) * 128], in_=ht[:])

        def gather(ch):
            nc.gpsimd.collective_compute(
                "AllGather",
                mybir.AluOpType.bypass,
                replica_groups=[list(range(NCORES))],
                ins=[h_tb[ch][:]],
                outs=[h_ag[ch][:]],
            )
            for r in range(NCORES):
                nc.gpsimd.dma_start(
                    out=stath[r][ch][:], in_=h_ag[ch][r * 128 : (r + 1) * 128, :]
                )

        layer(lhs_l0, w0_sb, write_l0, gather, piece_outer=False)

        # ---- layer 1 ----
        def write_l1(ch, it, lp):
            ht = h_pool.tile([128, D], F32, name="ht1", tag="ht1")
            nc.scalar.activation(ht[:], lp[:], relu)
            nc.scalar.dma_start(
                out=h_out[ch * IC + it * 128 : ch * IC + (it + 1) * 128, :], in_=ht[:]
            )

        layer(lhs_l1, w1_sb, write_l1, lambda ch: None, piece_outer=True)

    nc.finalize()
    return nc


def _tile_stat(X):
    """[16384, 128] -> [1024, 2048] stationary layout (fp32 in, fp32 out)."""
    return np.ascontiguousarray(
        X.reshape(NCORES, 16, 128, D).transpose(0, 2, 1, 3).reshape(NCORES * 128, ROWS)
    )


def shard_inputs(A_norm, X, W0, W1):
    """Host-side shard prep. Returns per-core input maps."""
    import ml_dtypes

    bf16 = ml_dtypes.bfloat16
    e3 = ml_dtypes.float8_e3m4

    x_t = _tile_stat(np.asarray(X, np.float32)).astype(bf16)
    w0 = np.ascontiguousarray(np.asarray(W0, np.float32) / np.float32(S_A))
    w1 = np.ascontiguousarray(np.asarray(W1, np.float32) / np.float32(S_A))

    in_maps = []
    for c in range(NCORES):
        a_tc = np.asarray(A_norm[c * ROWS : (c + 1) * ROWS, :], np.float32).T
        a8 = np.clip(a_tc * np.float32(S_A), 0.0, 15.5).astype(e3)
        # [16384, 2048] -> chunk-major groups (see a_in comment)
        a_pre = np.ascontiguousarray(
            a8.reshape(NCORES, NPIECE, NT, 128, NCH, IC)
            .transpose(4, 1, 3, 0, 2, 5)
            .reshape(NG * 128, GW)
        )
        in_maps.append({"a0": a_pre, "x0": x_t, "w0": w0, "w1": w1})
    return in_maps


_CACHED = {}


def kernel(A_norm, X, W0, W1):
    A_norm = np.ascontiguousarray(A_norm, dtype=np.float32)
    X = np.ascontiguousarray(X, dtype=np.float32)
    W0 = np.ascontiguousarray(W0, dtype=np.float32)
    W1 = np.ascontiguousarray(W1, dtype=np.float32)

    from concourse.bass_utils import run_bass_kernel_spmd

    if PRECISION not in _CACHED:
        _CACHED[PRECISION] = build_gcn()
    nc = _CACHED[PRECISION]

    in_maps = shard_inputs(A_norm, X, W0, W1)
    res = run_bass_kernel_spmd(nc, in_maps, core_ids=list(range(NCORES)))
    return np.concatenate([res.results[c]["h_out"] for c in range(NCORES)], axis=0)


# revision 9
# speedup vs baseline: 1.7308x; 1.1614x over previous
"""2-layer dense GCN on 8 Trainium2 NeuronCores — fp8 A-stream + DoubleRow.

Reference computation (all fp32):
    H0 = relu((A_norm @ X) @ W0)
    H1 = relu((A_norm @ H0) @ W1)
A_norm: [16384, 16384], X: [16384, 128], W0/W1: [128, 128].

Sharding: 1D row partition of A_norm (2048 rows/core). Each core holds
A[rows_c].T host-quantized to fp8-e4m3 (x 2^16 scale, folded back via
the W matrices), streamed over HBM at 1 byte/element — half the DMA of
bf16, which is the dominant traffic.

Precision scheme (sim: rel err 2.3e-3 vs 2e-2 budget):
  layer 0: stationary X bf16  x  moving A e4m3   (mixed-dtype matmul)
  layer 1: stationary H e4m3  x  moving A e4m3   (DoubleRow, 2 MACs/cell)
X quantized to fp8 would cost ~1.5e-2 (zero-mean cancellation amplifies
quantization noise) so X stays bf16; H is post-relu/positive and cheap
to quantize, enabling DoubleRow's 2x PE rate for the whole second layer.

Schedule (per layer, per core): the 2048 output rows split into 4
chunks of 512 (one PSUM bank each). A^T is host-tiled chunk-major so
layer 0 finishes chunk k at ~(k+1)/4 of the layer and AllGathers it
immediately, overlapping remaining compute. Layer 1 runs piece-outer
(all chunks' piece-p matmuls per round) so gather chunk p is only
needed at round p — each gather gets a ~1/4-layer deadline slack and
never stalls the PE.

Aggregate matmul:  psum[d, i] += H_tile[q, d].T @ A^T[q, i]
Linear matmul:     psum[i, e]  = M[d, i-slice].T @ W[d, e]  (fp32r)
Relu fused into the PSUM eviction on the scalar engine (H scale 2^8
folded into W0 so the bf16->e4m3 convert is a plain relu+cast).
"""

import sys
from contextlib import ExitStack

if "/opt/trn_rl_repo" not in sys.path:
    sys.path.insert(0, "/opt/trn_rl_repo")

import numpy as np

N_NODES = 16384
D = 128
NCORES = 8
ROWS = N_NODES // NCORES   # 2048 output rows per core
NCH = 4                    # output column chunks (512 wide, 1 PSUM bank)
NPIECE = 4                 # j-piece blocks per rank (gather granularity)
NT = 4                     # j-tiles per piece
IC = 512                   # chunk width
NG = NCH * NPIECE          # A DMA groups per layer (2 MiB each)
NB = NCORES * NT           # j-tile blocks per A group
S_A = float(2 ** 16)       # fp8 pre-scale for A (max |A|*S_A ~ 8.1)
S_H = float(2 ** 8)        # fp8 pre-scale for H (max ~12)

PRECISION = "fp8dr"


def build_gcn():
    """Build the SPMD Bass program (one program, runs on all cores)."""
    import concourse.bass as bass  # noqa: F401
    import concourse.tile as tile
    from concourse import bacc, mybir

    F32 = mybir.dt.float32
    F32R = mybir.dt.float32r
    BF16 = mybir.dt.bfloat16
    E4 = mybir.dt.float8e4
    relu = mybir.ActivationFunctionType.Relu
    DR = mybir.MatmulPerfMode.DoubleRow

    nc = bacc.Bacc("TRN2", target_bir_lowering=False, num_devices=NCORES)

    # A^T shard, host pre-tiled chunk-major (see shard_inputs):
    # group g=(ch*NPIECE+p) is rows [g*128, (g+1)*128) with
    # a_in[g*128+q, (r*NT+t)*IC + i] = A^T[(r*16+p*NT+t)*128+q, ch*IC+i]
    a_in = nc.dram_tensor("a0", [NG * 128, NB * IC], E4, kind="ExternalInput")
    # x_in[r*128+p, tl*128+dd] = X[(r*16+tl)*128+p, dd]  (bf16)
    x_in = nc.dram_tensor("x0", [NCORES * 128, ROWS], BF16, kind="ExternalInput")
    w0 = nc.dram_tensor("w0", [D, D], F32R, kind="ExternalInput")
    w1 = nc.dram_tensor("w1", [D, D], F32R, kind="ExternalInput")
    h_out = nc.dram_tensor("h_out", [ROWS, D], F32, kind="ExternalOutput")

    with tile.TileContext(nc) as tc, ExitStack() as ctx:
        sb1 = ctx.enter_context(tc.tile_pool(name="sb1", bufs=1))
        statx_pool = ctx.enter_context(tc.tile_pool(name="sx", bufs=NCORES))
        stath_pool = ctx.enter_context(
            tc.tile_pool(name="sh", bufs=NCORES * NPIECE)
        )
        a_pool = ctx.enter_context(tc.tile_pool(name="a", bufs=6))
        m_pool = ctx.enter_context(tc.tile_pool(name="m", bufs=2))
        h_pool = ctx.enter_context(tc.tile_pool(name="h", bufs=4))
        agg_pool = ctx.enter_context(tc.tile_pool(name="agg", bufs=4, space="PSUM"))
        lin_pool = ctx.enter_context(tc.tile_pool(name="lin", bufs=2, space="PSUM"))
        dram = ctx.enter_context(tc.tile_pool(name="dram", bufs=1, space="DRAM"))

        w0_sb = sb1.tile([D, D], F32R)
        nc.scalar.dma_start(out=w0_sb[:], in_=w0[:])
        w1_sb = sb1.tile([D, D], F32R)
        nc.scalar.dma_start(out=w1_sb[:], in_=w1[:])

        # stationary X on the fast hwdge queues, ahead of the A stream
        # (needed first; keeps gpsimd free for h writes + gather triggers)
        statx = []
        for r in range(NCORES):
            t_ = statx_pool.tile([128, ROWS], BF16, name=f"sx{r}", tag="sx")
            eng = nc.sync if r % 2 == 0 else nc.scalar
            eng.dma_start(out=t_[:], in_=x_in[r * 128 : (r + 1) * 128, :])
            statx.append(t_)

        # hidden-state bounce + gather buffers, one per chunk (e4m3)
        h_tb = [dram.tile([128, IC], E4, name=f"htb{c}") for c in range(NCH)]
        h_ag = [
            dram.tile([NCORES * 128, IC], E4, addr_space="Shared", name=f"hag{c}")
            for c in range(NCH)
        ]
        # gathered H as stationary tiles, one per (rank, piece); 3D so
        # DoubleRow can take k-subtile pairs on dim 1
        stath = [
            [
                stath_pool.tile([128, NT, 128], E4, name=f"sh{r}_{p}", tag="sh")
                for p in range(NPIECE)
            ]
            for r in range(NCORES)
        ]

        def layer(w_sb, write_out, chunk_done, piece_outer, double_row):
            # schedule: L0 chunk-outer (finish chunks early -> early gathers);
            # L1 piece-outer (piece-p rounds start ~1/4-layer apart, so each
            # gather chunk has a late deadline and never stalls the PE)
            if piece_outer:
                sched = [(ch, p) for p in range(NPIECE) for ch in range(NCH)]
            else:
                sched = [(ch, p) for ch in range(NCH) for p in range(NPIECE)]
            aggs = {}
            neng = 0
            for ch, p in sched:
                if p == 0:
                    aggs[ch] = agg_pool.tile([128, IC], F32, name=f"agg{ch}", tag="agg")
                agg = aggs[ch]
                g = ch * NPIECE + p
                at = a_pool.tile([128, NB, IC], E4, name="at", tag="at")
                eng = nc.sync if neng % 2 == 0 else nc.scalar
                neng += 1
                eng.dma_start(out=at[:], in_=a_in[g * 128 : (g + 1) * 128, :])
                if double_row:
                    for r in range(NCORES):
                        for u in range(NT // 2):
                            nc.tensor.matmul(
                                agg[:],
                                lhsT=stath[r][p][:, 2 * u : 2 * u + 2, :],
                                rhs=at[:, r * NT + 2 * u : r * NT + 2 * u + 2, :],
                                start=(p == 0 and r == 0 and u == 0),
                                stop=(
                                    p == NPIECE - 1
                                    and r == NCORES - 1
                                    and u == NT // 2 - 1
                                ),
                                perf_mode=DR,
                            )
                else:
                    for r in range(NCORES):
                        for t in range(NT):
                            jr = p * NT + t
                            nc.tensor.matmul(
                                agg[:],
                                lhsT=statx[r][:, jr * 128 : (jr + 1) * 128],
                                rhs=at[:, r * NT + t : r * NT + t + 1, :],
                                start=(p == 0 and r == 0 and t == 0),
                                stop=(
                                    p == NPIECE - 1
                                    and r == NCORES - 1
                                    and t == NT - 1
                                ),
                            )
                if p == NPIECE - 1:
                    mt = m_pool.tile([128, IC], F32R, name="mt", tag="mt")
                    nc.vector.tensor_copy(out=mt[:], in_=agg[:])
                    for it in range(IC // 128):
                        lp = lin_pool.tile([128, D], F32, name="lp", tag="lp")
                        nc.tensor.matmul(
                            lp[:],
                            lhsT=mt[:, it * 128 : (it + 1) * 128],
                            rhs=w_sb[:],
                            start=True,
                            stop=True,
                        )
                        write_out(ch, it, lp)
                    chunk_done(ch)

        # ---- layer 0 ----
        def write_l0(ch, it, lp):
            ht = h_pool.tile([128, D], E4, name="ht0", tag="ht0")
            nc.scalar.activation(ht[:], lp[:], relu)
            # gpsimd queue: keeps the latency-sensitive h write off the
            # A-stream queues so the gather triggers early
            nc.gpsimd.dma_start(out=h_tb[ch][:, it * 128 : (it + 1) * 128], in_=ht[:])

        def gather(ch):
            nc.gpsimd.collective_compute(
                "AllGather",
                mybir.AluOpType.bypass,
                replica_groups=[list(range(NCORES))],
                ins=[h_tb[ch][:]],
                outs=[h_ag[ch][:]],
            )
            for r in range(NCORES):
                nc.gpsimd.dma_start(
                    out=stath[r][ch][:], in_=h_ag[ch][r * 128 : (r + 1) * 128, :]
                )

        layer(w0_sb, write_l0, gather, piece_outer=False, double_row=False)

        # ---- layer 1 ----
        def write_l1(ch, it, lp):
            ht = h_pool.tile([128, D], F32, name="ht1", tag="ht1")
            nc.scalar.activation(ht[:], lp[:], relu)
            nc.scalar.dma_start(
                out=h_out[ch * IC + it * 128 : ch * IC + (it + 1) * 128, :], in_=ht[:]
            )

        layer(w1_sb, write_l1, lambda ch: None, piece_outer=True, double_row=True)

    nc.finalize()
    return nc


def _tile_stat(X):
    """[16384, 128] -> [1024, 2048] stationary layout."""
    return np.ascontiguousarray(
        X.reshape(NCORES, 16, 128, D).transpose(0, 2, 1, 3).reshape(NCORES * 128, ROWS)
    )


def shard_inputs(A_norm, X, W0, W1):
    """Host-side shard prep. Returns per-core input maps."""
    import ml_dtypes

    bf16 = ml_dtypes.bfloat16
    e4 = ml_dtypes.float8_e4m3

    x_t = _tile_stat(np.asarray(X, np.float32)).astype(bf16)
    # fold the fp8 pre-scales into the weights:
    #   psum_l0 = S_A*(A@X);      w0' = W0*S_H/S_A  -> h_tb = S_H*H0 (e4m3)
    #   psum_l1 = S_A*S_H*(A@H0); w1' = W1/(S_A*S_H)
    w0 = np.ascontiguousarray(np.asarray(W0, np.float32) * np.float32(S_H / S_A))
    w1 = np.ascontiguousarray(np.asarray(W1, np.float32) / np.float32(S_A * S_H))

    in_maps = []
    for c in range(NCORES):
        a_tc = np.asarray(A_norm[c * ROWS : (c + 1) * ROWS, :], np.float32).T
        a8 = np.clip(a_tc * np.float32(S_A), 0.0, 240.0).astype(e4)
        # [16384, 2048] -> chunk-major groups (see a_in comment)
        a_pre = np.ascontiguousarray(
            a8.reshape(NCORES, NPIECE, NT, 128, NCH, IC)
            .transpose(4, 1, 3, 0, 2, 5)
            .reshape(NG * 128, NB * IC)
        )
        in_maps.append({"a0": a_pre, "x0": x_t, "w0": w0, "w1": w1})
    return in_maps


_CACHED = {}


def kernel(A_norm, X, W0, W1):
    A_norm = np.ascontiguousarray(A_norm, dtype=np.float32)
    X = np.ascontiguousarray(X, dtype=np.float32)
    W0 = np.ascontiguousarray(W0, dtype=np.float32)
    W1 = np.ascontiguousarray(W1, dtype=np.float32)

    from concourse.bass_utils import run_bass_kernel_spmd

    if PRECISION not in _CACHED:
        _CACHED[PRECISION] = build_gcn()
    nc = _CACHED[PRECISION]

    in_maps = shard_inputs(A_norm, X, W0, W1)
    res = run_bass_kernel_spmd(nc, in_maps, core_ids=list(range(NCORES)))
    return np.concatenate([res.results[c]["h_out"] for c in range(NCORES)], axis=0)
